# revision 21
# baseline (speedup 1.0000x reference)
"""MiniGPT (dense transformer) Trainium2 Bass kernel — v3.

Sharding: 8 cores = 4 sequences (DP) x TP-2.
  core c: seq = c//2, tp = c%2.
  TP-2: heads 6+6 (QKV column / O row parallel), FFN (w1 col / w2 row),
  vocab-parallel lm_head. Pairwise AllReduce after O-proj and FFN2,
  emitted at T-half granularity so collectives overlap the other half.

v3 changes vs v2 (scheduling/collective round):
  * AllReduce payloads bf16 (halves CC latency + DRAM bounce traffic);
    gpsimd software-DGE does the cast+accumulate back into the f32
    residual in one DMA.
  * LN chains restructured: stats via ACT accumulate (Identity-sum pass,
    Square-accum pass centered with a per-partition -mean bias), rstd via
    exp(-0.5*ln(var+eps)) — everything in ACT's natural_log_exp table
    set. Chains are emitted one phase EARLY and hosted on whichever
    engine idles in their window (full-ACT during ffn/lm windows,
    DVE-stats hybrid inside attention), so the ~10us serial LN chain at
    every phase boundary overlaps the previous phase's PE work.
  * causal diag masking moved off gpsimd (affine_select) onto a DVE
    multiply with a precomputed triangle mask — the gpsimd queue used to
    serialize pairs' masking behind collective waits, starving the PE
    for ~30us per layer.
  * exp only over the causally-valid score columns.
  * next layer's wqkv/wo/w1 DMAs emitted a phase early (prefetch).
  * scheduler fences (no_sync_barrier) removed so independent PE work
    can fill exp/normalize bubbles.
  * lm_head: first 8 vocab tiles run their half-0 token chunks under the
    last AllReduce (weights re-streamed for half 1), lm PSUM->SBUF
    copies forced to DVE in that window; no pool close (kills a 12.6us
    gpsimd DRAIN at lm entry).

Layouts (per core):
  residual x:  SBUF [128, 8, 768] f32 token-major (part=t%128, chunk t//128)
  h^T:         SBUF [128, 6, 512] bf16 per T-half (part=d%128, ktile d//128)
  q^T,k^T:     SBUF [128, 512] bf16 per (pair, half); head hh at parts hh*64..
  v:           SBUF [128, 4, 6, 65] bf16 natural [j, head, d'] + ones column
  es:          SBUF [128, njt, 2, 512] bf16 exp'd scores (hh interleaved)
  attn oT:     SBUF [64, 512] bf16 per (pair, hh), normalized
  PSUM: sc [128,2,512]x2 (scores pairs / O-proj / FFN2 out),
        po [128,512]x2 (PV + FFN1 u), misc [128,512]x2 (QKV/LNT/lm).
"""

import sys
import numpy as np

for _p in ("/opt/trn_rl_repo",):
    if _p not in sys.path:
        sys.path.insert(0, _p)

import ml_dtypes
import concourse.bass as bass
import concourse.tile as tile
from concourse import bacc, mybir
from concourse import bass_utils
from concourse.masks import make_identity
from contextlib import ExitStack

F32 = mybir.dt.float32
F32R = mybir.dt.float32r
BF16 = mybir.dt.bfloat16
AF = mybir.ActivationFunctionType
ALU = mybir.AluOpType

V, D, H, L, T, B = 32000, 768, 12, 4, 1024, 4
HD = D // H            # 64
NCORES = 8
TP = 2
DL = D // TP           # 384 local head dims (6 heads)
LH = H // TP           # 6 local heads
F1 = 4 * D // TP       # 1536 local ffn dim
VL = V // TP           # 16000 local vocab
P = 128
NT = T // P            # 8 token chunks
TQH = 512              # T-half
KD = D // P            # 6

AR_BF16 = True         # collective payload dtype
LM_FIRST = 8           # vocab tiles whose half-0 chunks run under the last AR


def _r(ap):
    return ap.bitcast(F32R)


def _pin_act_table():
    """Restrict the activation-table chooser to the one set that holds
    every function this kernel uses (exp/ln/identity/square/copy), so the
    compiled stream has a single ACT_TABLE_LOAD instead of thrashing
    between the exp-only and ln-only sets (1.28us per reload, in the
    latency-critical LN chains)."""
    import concourse.bacc as bacc_mod
    orig = bacc_mod.get_activation_tables
    if getattr(orig, "_pinned_nle", False):
        return
    def pinned(arch):
        t = orig(arch)
        name = "natural_log_exp_and_others"
        if name not in t:
            return t
        # keep dict order/indices (act_func_set_id is positional) but make
        # every other set unusable so the chooser always lands on `name`
        return {k: (v if k == name else set()) for k, v in t.items()}
    pinned._pinned_nle = True
    bacc_mod.get_activation_tables = pinned


def build_program(bias_flags):
    _pin_act_table()
    nc = bacc.Bacc(
        "TRN2",
        target_bir_lowering=False,
        debug=False,
        enable_asserts=False,
        num_devices=NCORES,
    )

    d = {}
    d["x0"] = nc.dram_tensor("x0", [T, D], F32, kind="ExternalInput").ap()
    d["h0T"] = nc.dram_tensor("h0T", [P, KD, T], BF16, kind="ExternalInput").ap()
    d["wqkv"] = nc.dram_tensor("wqkv", [L, D, 3 * DL], BF16, kind="ExternalInput").ap()
    d["wo"] = nc.dram_tensor("wo", [L, P, LH // 2, D], BF16, kind="ExternalInput").ap()
    d["w1"] = nc.dram_tensor("w1", [L, D, F1], BF16, kind="ExternalInput").ap()
    d["w2"] = nc.dram_tensor("w2", [L, F1, D], BF16, kind="ExternalInput").ap()
    d["wlm"] = nc.dram_tensor("wlm", [D, VL], BF16, kind="ExternalInput").ap()
    d["bqkv"] = nc.dram_tensor("bqkv", [L, 3 * DL], F32, kind="ExternalInput").ap()
    d["bo"] = nc.dram_tensor("bo", [L, D], F32, kind="ExternalInput").ap()
    d["b1"] = nc.dram_tensor("b1", [L, F1], F32, kind="ExternalInput").ap()
    d["b2"] = nc.dram_tensor("b2", [L, D], F32, kind="ExternalInput").ap()
    d["blm"] = nc.dram_tensor("blm", [VL], F32, kind="ExternalInput").ap()
    d["out"] = nc.dram_tensor("logits", [T, VL], BF16, kind="ExternalOutput").ap()

    with tile.TileContext(nc) as tc, ExitStack() as ctx:
        _body(ctx, tc, bias_flags, d)
    nc.compile()
    return nc


def _body(ctx, tc, bf, d):
    nc = tc.nc
    pool = lambda name, bufs, **kw: ctx.enter_context(
        tc.tile_pool(name=name, bufs=bufs, **kw))

    const = pool("const", 1)
    ln_p = pool("ln", 4)
    lnh_p = pool("lnh", 8)
    sq_p = pool("sq", 2)
    x_p = pool("x", 1)
    hT_p = pool("hT", 2)
    lmw_p = pool("lmw", 2)
    lmo_p = pool("lmo", 4)
    dram = pool("dram", 16, space="DRAM")

    q_p = pool("q", 4)
    k_p = pool("k", 6)
    v_p = pool("v", 2)
    es_p = pool("es", 2)
    oT_p = pool("oT", 8)
    rdn_p = pool("rdn", 4)
    y_p = pool("y", 2)
    um_p = pool("um", 3)
    wqkv_p = pool("wqkv", 8)
    wo_p = pool("wo", 2)
    w1_p = pool("w1", 7)
    w2_p = pool("w2", 3)
    bias_p = pool("bias", 2)

    # PSUM: 8 banks total
    sc_p = pool("sc", 2, space="PSUM")       # [128, 2, 512] f32 = 2 banks each
    po_p = pool("po", 2, space="PSUM")       # [128, 512] 1 bank each
    misc_p = pool("misc", 2, space="PSUM")   # [128, 512] 1 bank each

    # constants
    ident = const.tile([P, P], BF16)
    make_identity(nc, ident)
    magic4 = const.tile([P, 4], mybir.dt.int32)
    nc.vector.memset(magic4, 0x5F3759DF)
    # pre-warm the ACT table set (exp/ln/identity/square share one set)
    warm = const.tile([P, 1], F32)
    nc.scalar.activation(warm, magic4[:, 0:1].bitcast(F32), AF.Exp, scale=0.0)
    ones64 = const.tile([P, HD], F32)
    nc.vector.memset(ones64, 1.0)
    epsc = const.tile([P, 1], F32)
    nc.vector.memset(epsc, 1e-5)
    # causal mask for the diagonal 128-chunk: keep t >= j, zero above
    trimask = const.tile([P, P], BF16)
    nc.vector.memset(trimask, 1.0)
    nc.gpsimd.affine_select(
        out=trimask, in_=trimask, compare_op=ALU.is_ge, fill=0.0,
        base=0, channel_multiplier=-1, pattern=[[1, P]])
    ones_row = None
    if any(bf.values()):
        ones_row = const.tile([1, P], F32)
        nc.vector.memset(ones_row, 1.0)

    x_sb = x_p.tile([P, NT, D], F32)

    # ---------------- LN chains ----------------
    # rstd = exp(-0.5 * ln(var + eps)); var from ACT accumulate passes.

    def chain_act(half):
        """Full-ACT LN chain for `half`: use when the chain's execution
        window has an idle ACT (ffn / lm entry windows)."""
        s4 = ln_p.tile([P, 2, 4], F32, tag="s4")
        negm4 = ln_p.tile([P, 4], F32, tag="negm4")
        for tcl in range(4):
            xc = x_sb[:, half * 4 + tcl, :]
            junk = sq_p.tile([P, D], BF16, tag="junk", name="junk")
            nc.scalar.activation(junk, xc, AF.Identity,
                                 accum_out=s4[:, 0, tcl:tcl + 1])
            nc.scalar.mul(negm4[:, tcl:tcl + 1], s4[:, 0, tcl:tcl + 1],
                          -1.0 / D)
            junk2 = sq_p.tile([P, D], BF16, tag="junk", name="junk2")
            nc.scalar.activation(junk2, xc, AF.Square,
                                 bias=negm4[:, tcl:tcl + 1],
                                 accum_out=s4[:, 1, tcl:tcl + 1])
        l4 = ln_p.tile([P, 4], F32, tag="l4")
        nc.scalar.activation(l4, s4[:, 1, :], AF.Ln, scale=1.0 / D, bias=epsc)
        rstd4 = ln_p.tile([P, 4], F32, tag="rstd4")
        nc.scalar.activation(rstd4, l4, AF.Exp, scale=-0.5)
        nm4 = ln_p.tile([P, 4], F32, tag="nm4")
        for tcl in range(4):
            nc.scalar.mul(nm4[:, tcl:tcl + 1], rstd4[:, tcl:tcl + 1],
                          negm4[:, tcl:tcl + 1])
        hts = []
        for tcl in range(4):
            h = lnh_p.tile([P, D], BF16, tag="h")
            nc.scalar.activation(
                h, x_sb[:, half * 4 + tcl, :], AF.Identity,
                bias=nm4[:, tcl:tcl + 1], scale=rstd4[:, tcl:tcl + 1])
            hts.append(h)
        return hts

    def chain_stats(half):
        """DVE bn_stats part of the hybrid chain (emit where DVE drains
        early relative to the accum feeding it)."""
        mv4 = ln_p.tile([P, 2, 4], F32, tag="mv4")
        for tcl in range(4):
            xc = x_sb[:, half * 4 + tcl, :]
            st = ln_p.tile([P, 2, 6], F32, tag="st")
            for s in range(2):
                nc.vector.bn_stats(st[:, s, :], xc[:, s * 384:(s + 1) * 384])
            nc.vector.bn_aggr(mv4[:, :, tcl], st)
        return mv4

    def chain_tail(mv4, half):
        """ACT tail of the hybrid chain."""
        l4 = ln_p.tile([P, 4], F32, tag="l4")
        nc.scalar.activation(l4, mv4[:, 1, :], AF.Ln, bias=epsc)
        rstd4 = ln_p.tile([P, 4], F32, tag="rstd4")
        nc.scalar.activation(rstd4, l4, AF.Exp, scale=-0.5)
        negm4 = ln_p.tile([P, 4], F32, tag="negm4")
        nc.scalar.mul(negm4, mv4[:, 0, :], -1.0)
        nm4 = ln_p.tile([P, 4], F32, tag="nm4")
        for tcl in range(4):
            nc.scalar.mul(nm4[:, tcl:tcl + 1], rstd4[:, tcl:tcl + 1],
                          negm4[:, tcl:tcl + 1])
        hts = []
        for tcl in range(4):
            h = lnh_p.tile([P, D], BF16, tag="h")
            nc.scalar.activation(
                h, x_sb[:, half * 4 + tcl, :], AF.Identity,
                bias=nm4[:, tcl:tcl + 1], scale=rstd4[:, tcl:tcl + 1])
            hts.append(h)
        return hts

    def ln_transpose(hts):
        hT = hT_p.tile([P, KD, TQH], BF16, tag="hT")
        for kt in range(KD):
            pt = misc_p.tile([P, TQH], F32, tag="misc", name="ptb").bitcast(BF16)
            for tcl in range(4):
                nc.tensor.transpose(
                    pt[:, tcl * P:(tcl + 1) * P],
                    hts[tcl][:, kt * P:(kt + 1) * P], ident)
            nc.vector.tensor_copy(hT[:, kt, :], pt[:, 0:TQH])
        return hT

    def bias_mm(psum_ap, brow_ap):
        # += ones^T @ brow : K=1 matmul accumulating a broadcast row vector
        nc.tensor.matmul(psum_ap, _r(ones_row), _r(brow_ap),
                         start=False, stop=False)

    # ---------------- layer weight loads ----------------
    wqkv_sbs = [None] * L
    wo_sbs = [None] * L
    w1_sbs = [None] * L
    bias_sbs = [None] * L

    def load_wqkv(l):
        ws = []
        for kt in range(KD):
            w = wqkv_p.tile([P, 3 * DL], BF16, tag="wqkv")
            nc.sync.dma_start(w, d["wqkv"][l, kt * P:(kt + 1) * P, :])
            ws.append(w)
        wqkv_sbs[l] = ws
        bqk_sb = brow_v = brow_o = brow_2 = b1_sb = None
        if bf["qk"]:
            bqk_sb = bias_p.tile([P, 6], F32, tag="bqk")
            nc.sync.dma_start(
                bqk_sb,
                d["bqkv"][l, 0:2 * DL].rearrange("(w q p) -> p (w q)", p=P, w=2))
        if bf["v"]:
            brow_v = bias_p.tile([1, DL], F32, tag="bv")
            nc.sync.dma_start(brow_v, d["bqkv"][l, 2 * DL:3 * DL][None, :])
        if bf["o"]:
            brow_o = bias_p.tile([1, D], F32, tag="bo")
            nc.sync.dma_start(brow_o, d["bo"][l][None, :])
        if bf["b1"]:
            b1_sb = bias_p.tile([P, 12], F32, tag="b1")
            nc.sync.dma_start(b1_sb, d["b1"][l].rearrange("(m p) -> p m", p=P))
        if bf["b2"]:
            brow_2 = bias_p.tile([1, D], F32, tag="b2")
            nc.sync.dma_start(brow_2, d["b2"][l][None, :])
        bias_sbs[l] = (bqk_sb, brow_v, brow_o, brow_2, b1_sb)

    def load_wo_w1(l):
        wo_sb = wo_p.tile([P, LH // 2, D], BF16, tag="wo")
        nc.sync.dma_start(wo_sb, d["wo"][l])
        wo_sbs[l] = wo_sb
        ws = []
        for kt in range(KD):
            w = w1_p.tile([P, F1], BF16, tag="w1")
            nc.sync.dma_start(w, d["w1"][l, kt * P:(kt + 1) * P, :])
            ws.append(w)
        w1_sbs[l] = ws

    # ---------------- transformer layers ----------------
    qT, kT, v_sb, oT = {}, {}, {}, {}

    def qkv_section(l, half, hT):
        wqkv_sb = wqkv_sbs[l]
        bqk_sb, brow_v = bias_sbs[l][0], bias_sbs[l][1]
        for pair in range(3):
            for which, store, pp in ((0, qT, q_p), (1, kT, k_p)):
                dst = pp.tile([P, TQH], BF16, tag="qkT")
                ps = misc_p.tile([P, TQH], F32, tag="misc")
                for kt in range(KD):
                    nc.tensor.matmul(
                        ps,
                        wqkv_sb[kt][:, which * DL + pair * P:
                                    which * DL + (pair + 1) * P],
                        hT[:, kt, :],
                        start=(kt == 0), stop=(kt == KD - 1))
                if bf["qk"]:
                    nc.scalar.activation(
                        dst, ps, AF.Identity,
                        bias=bqk_sb[:, which * 3 + pair:which * 3 + pair + 1])
                else:
                    nc.vector.tensor_copy(dst, ps)
                store[(pair, half)] = dst
        # v natural [j, head, d'] + ones column, bf16
        vt = v_p.tile([P, 4, LH, HD + 1], BF16, tag="v")
        nc.vector.memset(vt[:, :, :, HD:HD + 1], 1.0)
        for jcl in range(4):
            ps = misc_p.tile([P, TQH], F32, tag="misc")
            for kt in range(KD):
                nc.tensor.matmul(
                    ps[:, 0:DL], hT[:, kt, jcl * P:(jcl + 1) * P],
                    wqkv_sb[kt][:, 2 * DL:3 * DL],
                    start=(kt == 0), stop=(kt == KD - 1))
            if bf["v"]:
                bias_mm(ps[:, 0:DL], brow_v)
            nc.vector.tensor_copy(
                vt[:, jcl, :, 0:HD],
                ps[:, 0:DL].rearrange("p (h e) -> p h e", h=LH))
        v_sb[half] = vt

    def pairs_section(half):
        # scores -> exp (merged across head pair) -> PV -> normalize
        njt = 4 * (half + 1)
        for pair in range(3):
            es = es_p.tile([P, 8, 2, TQH], BF16, tag="es")
            for jt in range(njt):
                lst = max(0, jt * P - half * TQH)
                sctile = sc_p.tile([P, 2, TQH], F32, tag="sc")
                for hh in range(2):
                    nc.tensor.matmul(
                        sctile[:, hh, lst:],
                        kT[(pair, jt // 4)][hh * HD:(hh + 1) * HD,
                                            (jt % 4) * P:(jt % 4 + 1) * P],
                        qT[(pair, half)][hh * HD:(hh + 1) * HD, lst:],
                        start=True, stop=True)
                nc.scalar.activation(es[:, jt, :, lst:], sctile[:, :, lst:],
                                     AF.Exp, scale=0.125)
                doff = jt * P - half * TQH
                if doff >= 0:
                    # zero the strictly-upper triangle of the diag chunk
                    nc.vector.tensor_mul(
                        es[:, jt, :, doff:doff + P],
                        es[:, jt, :, doff:doff + P],
                        trimask[:, None, :].broadcast_to((P, 2, P)))
            ot = oT_p.tile([P, TQH], BF16, tag="oT")
            ptb2 = misc_p.tile([P, TQH], F32, tag="misc",
                               name="ptm2").bitcast(BF16)
            for hh in range(2):
                lh = pair * 2 + hh
                po = po_p.tile([P, TQH], F32, tag="po")
                for jt in range(njt):
                    lst = max(0, jt * P - half * TQH)
                    nc.tensor.matmul(
                        po[0:HD + 1, lst:],
                        v_sb[jt // 4][:, jt % 4, lh, :],
                        es[:, jt, hh, lst:],
                        start=(jt == 0), stop=(jt == njt - 1))
                # normalize per token via transpose round-trip (bf16),
                # batched over the 4 token chunks.
                oT65 = rdn_p.tile([HD + 1, TQH], BF16, tag="oT65")
                nc.vector.tensor_copy(oT65, po[0:HD + 1, :])
                ptb = misc_p.tile([P, TQH], F32, tag="misc",
                                  name="ptm").bitcast(BF16)
                # stride 68 keeps each chunk's PSUM offset 4B-aligned
                ptv = ptb[:, 0:4 * 68].rearrange("p (a b) -> p a b", a=4)
                for tcl in range(4):
                    nc.tensor.transpose(
                        ptv[:, tcl, 0:HD + 1],
                        oT65[:, tcl * P:(tcl + 1) * P],
                        ident[0:HD + 1, 0:HD + 1])
                rc4 = ln_p.tile([P, 4], F32, tag="rc4")
                nc.vector.reciprocal(rc4, ptv[:, :, HD])
                on4 = rdn_p.tile([P, 4, HD], BF16, tag="on4")
                nc.vector.tensor_mul(
                    on4, ptv[:, :, 0:HD],
                    rc4[:, :, None].broadcast_to((P, 4, HD)))
                h0 = hh * HD
                for tcl in range(4):
                    nc.tensor.transpose(
                        ptb2[h0:h0 + HD, tcl * P:(tcl + 1) * P],
                        on4[:, tcl, :], ident)
            nc.vector.tensor_copy(ot, ptb2[:, 0:TQH])
            oT[pair] = ot

    AR_DT = BF16 if AR_BF16 else F32

    def do_allreduce(b_in, b_out, half):
        nc.gpsimd.collective_compute(
            "AllReduce", ALU.add,
            replica_groups=[[0, 1], [2, 3], [4, 5], [6, 7]],
            ins=[b_in.opt()], outs=[b_out.opt()])
        nc.gpsimd.dma_start(
            out=x_sb[:, half * 4:half * 4 + 4, :],
            in_=b_out.rearrange("(n p) t -> p n t", p=P),
            accum_op=ALU.add)

    def oproj_ar(l, half):
        # O-projection -> bounce -> AllReduce -> x += result
        wo_sb = wo_sbs[l]
        brow_o = bias_sbs[l][2]
        b_in = dram.tile([TQH, D], AR_DT, tag="bnc", name="b_in")
        b_out = dram.tile([TQH, D], AR_DT, tag="bnc", name="b_out")
        for tcl in range(4):
            py = sc_p.tile([P, 2, TQH], F32, tag="sc")
            pyf = py.rearrange("p a b -> p (a b)")
            for pairi in range(3):
                for n0, nw in ((0, 512), (512, 256)):
                    nc.tensor.matmul(
                        pyf[:, n0:n0 + nw],
                        oT[pairi][:, tcl * P:(tcl + 1) * P],
                        wo_sb[:, pairi, n0:n0 + nw],
                        start=(pairi == 0), stop=(pairi == 2))
            if bf["o"]:
                for n0, nw in ((0, 512), (512, 256)):
                    bias_mm(pyf[:, n0:n0 + nw], brow_o[:, n0:n0 + nw])
            ysb = y_p.tile([P, D], AR_DT, tag="y")
            nc.vector.tensor_copy(ysb, pyf[:, 0:D])
            nc.sync.dma_start(b_in[tcl * P:(tcl + 1) * P, :], ysb)
        do_allreduce(b_in, b_out, half)

    def ffn_half(l, half, hT2):
        w1_sb = w1_sbs[l]
        brow_2, b1_sb = bias_sbs[l][3], bias_sbs[l][4]
        b_in = dram.tile([TQH, D], AR_DT, tag="bnc", name="b_in")
        b_out = dram.tile([TQH, D], AR_DT, tag="bnc", name="b_out")
        for quarter in range(2):
            py0 = sc_p.tile([P, 2, TQH], F32, tag="sc")
            py1 = sc_p.tile([P, 2, TQH], F32, tag="sc")
            pyfs = [py0.rearrange("p a b -> p (a b)"),
                    py1.rearrange("p a b -> p (a b)")]
            for m in range(12):
                pu = po_p.tile([P, TQH], F32, tag="po")
                for kt in range(KD):
                    nc.tensor.matmul(
                        pu[:, 0:256], w1_sb[kt][:, m * P:(m + 1) * P],
                        hT2[:, kt, quarter * 256:(quarter + 1) * 256],
                        start=(kt == 0), stop=(kt == KD - 1))
                um = um_p.tile([P, 256], BF16, tag="uT")
                if bf["b1"]:
                    nc.vector.tensor_scalar(
                        um, pu[:, 0:256], b1_sb[:, m:m + 1], 0.0,
                        op0=ALU.add, op1=ALU.max)
                else:
                    nc.vector.tensor_scalar_max(um, pu[:, 0:256], 0.0)
                w2m = w2_p.tile([P, D], BF16, tag="w2")
                nc.sync.dma_start(w2m, d["w2"][l, m * P:(m + 1) * P, :])
                for t2 in range(2):
                    for n0, nw in ((0, 512), (512, 256)):
                        nc.tensor.matmul(
                            pyfs[t2][:, n0:n0 + nw],
                            um[:, t2 * P:(t2 + 1) * P],
                            w2m[:, n0:n0 + nw],
                            start=(m == 0), stop=(m == 11))
            for t2 in range(2):
                if bf["b2"]:
                    for n0, nw in ((0, 512), (512, 256)):
                        bias_mm(pyfs[t2][:, n0:n0 + nw], brow_2[:, n0:n0 + nw])
                ysb = y_p.tile([P, D], AR_DT, tag="y")
                nc.vector.tensor_copy(ysb, pyfs[t2][:, 0:D])
                tcl = quarter * 2 + t2
                nc.sync.dma_start(b_in[tcl * P:(tcl + 1) * P, :], ysb)
            # half-AR per quarter: chunk A leaves ~14us before the full
            # ffn half finishes, so the downstream LN chain's first token
            # chunks unblock a whole quarter earlier (subtile deps on x_sb
            # let the chain's per-tcl passes start individually).
            q0 = quarter * 256
            nc.gpsimd.collective_compute(
                "AllReduce", ALU.add,
                replica_groups=[[0, 1], [2, 3], [4, 5], [6, 7]],
                ins=[b_in[q0:q0 + 256, :].opt()],
                outs=[b_out[q0:q0 + 256, :].opt()])
            nc.gpsimd.dma_start(
                out=x_sb[:, half * 4 + 2 * quarter:half * 4 + 2 * quarter + 2, :],
                in_=b_out[q0:q0 + 256, :].rearrange("(n p) t -> p n t", p=P),
                accum_op=ALU.add)

    # ---------------- lm_head helpers ----------------
    brow_lm = None
    if bf["lm"]:
        brow_lm = lmo_p.tile([1, VL], F32, tag="blm")
        nc.sync.dma_start(brow_lm, d["blm"][None, :])
    nvt = (VL + 511) // 512
    hfT = [None, None]

    def lm_tile(vt, wt, tcgs, dve_only=False):
        v0 = vt * 512
        vw = min(512, VL - v0)
        for tcg in tcgs:
            half, tcl = tcg // 4, tcg % 4
            pl = misc_p.tile([P, 512], F32, tag="misc", name="pl")
            for kt in range(KD):
                nc.tensor.matmul(
                    pl[:, 0:vw],
                    hfT[half][:, kt, tcl * P:(tcl + 1) * P],
                    wt[:, kt, 0:vw],
                    start=(kt == 0), stop=(kt == KD - 1))
            if bf["lm"]:
                bias_mm(pl[:, 0:vw], brow_lm[:, v0:v0 + vw])
            lo = lmo_p.tile([P, 512], BF16, tag="lmo")
            if dve_only or tcg % 2 == 1:
                nc.vector.tensor_copy(lo[:, 0:vw], pl[:, 0:vw])
            else:
                nc.scalar.activation(lo[:, 0:vw], pl[:, 0:vw], AF.Copy)
            nc.sync.dma_start(
                d["out"][tcg * P:(tcg + 1) * P, v0:v0 + vw], lo[:, 0:vw])

    def lm_wt(vt):
        v0 = vt * 512
        vw = min(512, VL - v0)
        wt = lmw_p.tile([P, KD, 512], BF16, tag="lmw")
        nc.sync.dma_start(
            wt[:, :, 0:vw],
            d["wlm"][:, v0:v0 + vw].rearrange("(k p) w -> p k w", p=P))
        return wt

    # ---------------- emission ----------------
    # layer-0 LN(h) comes precomputed+pretransposed from the host: the
    # startup chain+transpose serial latency disappears entirely. Its DMA
    # goes first (with wqkv) so qkv(0) starts as early as possible; the
    # raw residual is only needed by the f0 chain, ~60us in.
    hT0in = [None, None]
    t_ = hT_p.tile([P, KD, TQH], BF16, tag="hT", name="hT0in0")
    nc.sync.dma_start(t_, d["h0T"][:, :, 0:TQH])
    hT0in[0] = t_
    load_wqkv(0)
    t_ = hT_p.tile([P, KD, TQH], BF16, tag="hT", name="hT0in1")
    nc.sync.dma_start(t_, d["h0T"][:, :, TQH:T])
    hT0in[1] = t_
    x0v = d["x0"].rearrange("(n p) t -> p n t", p=P)
    nc.sync.dma_start(x_sb[:, 0:4, :], x0v[:, 0:4, :])
    nc.sync.dma_start(x_sb[:, 4:8, :], x0v[:, 4:8, :])
    load_wo_w1(0)
    hts_a0 = None
    for l in range(L):
        hTa0 = hT0in[0] if l == 0 else ln_transpose(hts_a0)
        qkv_section(l, 0, hTa0)
        if l > 0:
            mv_a1 = chain_stats(1)  # dep: accum(f1^{l-1})
        pairs_section(0)
        if l > 0:
            hts_a1 = chain_tail(mv_a1, 1)
        oproj_ar(l, 0)
        hTa1 = hT0in[1] if l == 0 else ln_transpose(hts_a1)
        qkv_section(l, 1, hTa1)
        if l < L - 1:
            load_wqkv(l + 1)       # prefetch next layer's QKV weights
        else:
            wts01 = [lm_wt(0), lm_wt(1)]
        pairs_section(1)
        oproj_ar(l, 1)
        mv_f0 = chain_stats(0)     # dep: accum(a0^l); after ysb(a1) casts
        hts_f0 = chain_tail(mv_f0, 0)
        hts_f1 = chain_act(1)      # dep: accum(a1^l); runs during ffn(0)
        hTf0 = ln_transpose(hts_f0)
        ffn_half(l, 0, hTf0)
        hts_a0 = chain_act(0)      # dep: accum(f0^l); next layer / lm half0
        if l < L - 1:
            load_wo_w1(l + 1)      # prefetch next layer's wo/w1
        hTf1 = ln_transpose(hts_f1)
        ffn_half(l, 1, hTf1)

    # ---------------- final LN + lm_head ----------------
    hts_lm1 = chain_act(1)         # dep: accum(f1^{L-1}); runs under lm half0
    hfT[0] = ln_transpose(hts_a0)
    for vt in range(LM_FIRST):
        wt = wts01[vt] if vt < 2 else lm_wt(vt)
        lm_tile(vt, wt, range(4), dve_only=True)
    hfT[1] = ln_transpose(hts_lm1)
    for vt in range(LM_FIRST):
        lm_tile(vt, lm_wt(vt), range(4, 8))
    for vt in range(LM_FIRST, nvt):
        lm_tile(vt, lm_wt(vt), range(8))


# ---------------------------------------------------------------------------
# host side
# ---------------------------------------------------------------------------

_CACHE = {}


def _get_program(bias_flags):
    key = tuple(sorted(bias_flags.items()))
    if key not in _CACHE:
        _CACHE[key] = build_program(bias_flags)
    return _CACHE[key]


def _bf16(a):
    return np.ascontiguousarray(a.astype(ml_dtypes.bfloat16))


def make_in_maps(idx, tok_emb, pos_emb, wq, wk, wv, wo, bo,
                 ln1_g, ln1_b, ln2_g, ln2_b, w1, b1, w2, b2,
                 lnf_g, lnf_b, w_lm, b_lm):
    f = lambda a: np.asarray(a, dtype=np.float32)
    idx = np.asarray(idx)
    tok_emb, pos_emb = f(tok_emb), f(pos_emb)
    wq, wk, wv, wo, bo = f(wq), f(wk), f(wv), f(wo), f(bo)
    ln1_g, ln1_b, ln2_g, ln2_b = f(ln1_g), f(ln1_b), f(ln2_g), f(ln2_b)
    w1, b1, w2, b2 = f(w1), f(b1), f(w2), f(b2)
    lnf_g, lnf_b, w_lm, b_lm = f(lnf_g), f(lnf_b), f(w_lm), f(b_lm)

    # fold LN affine into following matmuls
    wq_f = ln1_g[:, :, None] * wq
    wk_f = ln1_g[:, :, None] * wk
    wv_f = ln1_g[:, :, None] * wv
    bq_f = np.einsum("ld,ldo->lo", ln1_b, wq)
    bk_f = np.einsum("ld,ldo->lo", ln1_b, wk)
    bv_f = np.einsum("ld,ldo->lo", ln1_b, wv)
    w1_f = ln2_g[:, :, None] * w1
    b1_f = b1 + np.einsum("ld,ldo->lo", ln2_b, w1)
    wlm_f = lnf_g[:, None] * w_lm
    blm_f = b_lm + lnf_b @ w_lm

    bias_flags = {
        "qk": bool(np.any(bq_f) or np.any(bk_f)),
        "v": bool(np.any(bv_f)),
        "o": bool(np.any(bo)),
        "b1": bool(np.any(b1_f)),
        "b2": bool(np.any(b2)),
        "lm": bool(np.any(blm_f)),
    }

    in_maps = []
    for c in range(NCORES):
        seq, tp = c // 2, c % 2
        sl = slice(tp * DL, (tp + 1) * DL)
        sf = slice(tp * F1, (tp + 1) * F1)
        sv = slice(tp * VL, (tp + 1) * VL)
        x0 = tok_emb[idx[seq]] + pos_emb[:T]
        m0 = x0.mean(-1, keepdims=True)
        v0 = x0.var(-1, keepdims=True)
        h0 = (x0 - m0) / np.sqrt(v0 + 1e-5)
        h0T = np.ascontiguousarray(
            h0.T.reshape(KD, P, T).transpose(1, 0, 2))
        wqkv_c = np.concatenate(
            [wq_f[:, :, sl], wk_f[:, :, sl], wv_f[:, :, sl]], axis=2)
        wo_c = np.ascontiguousarray(
            wo[:, sl, :].reshape(L, 3, 2, HD, D).transpose(0, 2, 3, 1, 4)
            .reshape(L, P, 3, D))
        bqkv_c = np.concatenate([bq_f[:, sl], bk_f[:, sl], bv_f[:, sl]], axis=1)
        in_maps.append({
            "x0": np.ascontiguousarray(x0, dtype=np.float32),
            "h0T": _bf16(h0T),
            "wqkv": _bf16(wqkv_c),
            "wo": _bf16(wo_c),
            "w1": _bf16(w1_f[:, :, sf]),
            "w2": _bf16(w2[:, sf, :]),
            "wlm": _bf16(wlm_f[:, sv]),
            "bqkv": np.ascontiguousarray(bqkv_c),
            "bo": np.ascontiguousarray(bo if tp == 0 else np.zeros_like(bo)),
            "b1": np.ascontiguousarray(b1_f[:, sf]),
            "b2": np.ascontiguousarray(b2 if tp == 0 else np.zeros_like(b2)),
            "blm": np.ascontiguousarray(blm_f[sv]),
        })
    return in_maps, bias_flags


def assemble(outs):
    logits = np.empty((B, T, V), dtype=np.float32)
    for seq in range(B):
        logits[seq, :, :VL] = np.asarray(outs[2 * seq], dtype=np.float32)
        logits[seq, :, VL:] = np.asarray(outs[2 * seq + 1], dtype=np.float32)
    return logits


def kernel(**inputs):
    in_maps, bias_flags = make_in_maps(**inputs)
    nc = _get_program(bias_flags)
    res = bass_utils.run_bass_kernel_spmd(
        nc, in_maps, core_ids=list(range(NCORES)))
    return assemble([res.results[c]["logits"] for c in range(NCORES)])


# revision 24
# speedup vs baseline: 1.0116x; 1.0116x over previous
"""MiniGPT (dense transformer) Trainium2 Bass kernel — v3.

Sharding: 8 cores = 4 sequences (DP) x TP-2.
  core c: seq = c//2, tp = c%2.
  TP-2: heads 6+6 (QKV column / O row parallel), FFN (w1 col / w2 row),
  vocab-parallel lm_head. Pairwise AllReduce after O-proj and FFN2,
  emitted at T-half granularity so collectives overlap the other half.

v3 changes vs v2 (scheduling/collective round):
  * AllReduce payloads bf16 (halves CC latency + DRAM bounce traffic);
    gpsimd software-DGE does the cast+accumulate back into the f32
    residual in one DMA.
  * LN chains restructured: stats via ACT accumulate (Identity-sum pass,
    Square-accum pass centered with a per-partition -mean bias), rstd via
    exp(-0.5*ln(var+eps)) — everything in ACT's natural_log_exp table
    set. Chains are emitted one phase EARLY and hosted on whichever
    engine idles in their window (full-ACT during ffn/lm windows,
    DVE-stats hybrid inside attention), so the ~10us serial LN chain at
    every phase boundary overlaps the previous phase's PE work.
  * causal diag masking moved off gpsimd (affine_select) onto a DVE
    multiply with a precomputed triangle mask — the gpsimd queue used to
    serialize pairs' masking behind collective waits, starving the PE
    for ~30us per layer.
  * exp only over the causally-valid score columns.
  * next layer's wqkv/wo/w1 DMAs emitted a phase early (prefetch).
  * scheduler fences (no_sync_barrier) removed so independent PE work
    can fill exp/normalize bubbles.
  * lm_head: first 8 vocab tiles run their half-0 token chunks under the
    last AllReduce (weights re-streamed for half 1), lm PSUM->SBUF
    copies forced to DVE in that window; no pool close (kills a 12.6us
    gpsimd DRAIN at lm entry).

Layouts (per core):
  residual x:  SBUF [128, 8, 768] f32 token-major (part=t%128, chunk t//128)
  h^T:         SBUF [128, 6, 512] bf16 per T-half (part=d%128, ktile d//128)
  q^T,k^T:     SBUF [128, 512] bf16 per (pair, half); head hh at parts hh*64..
  v:           SBUF [128, 4, 6, 65] bf16 natural [j, head, d'] + ones column
  es:          SBUF [128, njt, 2, 512] bf16 exp'd scores (hh interleaved)
  attn oT:     SBUF [64, 512] bf16 per (pair, hh), normalized
  PSUM: sc [128,2,512]x2 (scores pairs / O-proj / FFN2 out),
        po [128,512]x2 (PV + FFN1 u), misc [128,512]x2 (QKV/LNT/lm).
"""

import sys
import numpy as np

for _p in ("/opt/trn_rl_repo",):
    if _p not in sys.path:
        sys.path.insert(0, _p)

import ml_dtypes
import concourse.bass as bass
import concourse.tile as tile
from concourse import bacc, mybir
from concourse import bass_utils
from concourse.masks import make_identity
from contextlib import ExitStack

F32 = mybir.dt.float32
F32R = mybir.dt.float32r
BF16 = mybir.dt.bfloat16
AF = mybir.ActivationFunctionType
ALU = mybir.AluOpType

V, D, H, L, T, B = 32000, 768, 12, 4, 1024, 4
HD = D // H            # 64
NCORES = 8
TP = 2
DL = D // TP           # 384 local head dims (6 heads)
LH = H // TP           # 6 local heads
F1 = 4 * D // TP       # 1536 local ffn dim
VL = V // TP           # 16000 local vocab
P = 128
NT = T // P            # 8 token chunks
TQH = 512              # T-half
KD = D // P            # 6

AR_BF16 = True         # collective payload dtype
LM_FIRST = 8           # vocab tiles whose half-0 chunks run under the last AR


def _r(ap):
    return ap.bitcast(F32R)


def _pin_act_table():
    """Restrict the activation-table chooser to the one set that holds
    every function this kernel uses (exp/ln/identity/square/copy), so the
    compiled stream has a single ACT_TABLE_LOAD instead of thrashing
    between the exp-only and ln-only sets (1.28us per reload, in the
    latency-critical LN chains)."""
    import concourse.bacc as bacc_mod
    orig = bacc_mod.get_activation_tables
    if getattr(orig, "_pinned_nle", False):
        return
    def pinned(arch):
        t = orig(arch)
        name = "natural_log_exp_and_others"
        if name not in t:
            return t
        # keep dict order/indices (act_func_set_id is positional) but make
        # every other set unusable so the chooser always lands on `name`
        return {k: (v if k == name else set()) for k, v in t.items()}
    pinned._pinned_nle = True
    bacc_mod.get_activation_tables = pinned


def build_program(bias_flags):
    _pin_act_table()
    nc = bacc.Bacc(
        "TRN2",
        target_bir_lowering=False,
        debug=False,
        enable_asserts=False,
        num_devices=NCORES,
    )

    d = {}
    d["x0"] = nc.dram_tensor("x0", [T, D], F32, kind="ExternalInput").ap()
    d["h0T"] = nc.dram_tensor("h0T", [P, KD, T], BF16, kind="ExternalInput").ap()
    d["wqkv"] = nc.dram_tensor("wqkv", [L, D, 3 * DL], BF16, kind="ExternalInput").ap()
    d["wo"] = nc.dram_tensor("wo", [L, P, LH // 2, D], BF16, kind="ExternalInput").ap()
    d["w1"] = nc.dram_tensor("w1", [L, D, F1], BF16, kind="ExternalInput").ap()
    d["w2"] = nc.dram_tensor("w2", [L, F1, D], BF16, kind="ExternalInput").ap()
    d["wlm"] = nc.dram_tensor("wlm", [D, VL], BF16, kind="ExternalInput").ap()
    d["bqkv"] = nc.dram_tensor("bqkv", [L, 3 * DL], F32, kind="ExternalInput").ap()
    d["bo"] = nc.dram_tensor("bo", [L, D], F32, kind="ExternalInput").ap()
    d["b1"] = nc.dram_tensor("b1", [L, F1], F32, kind="ExternalInput").ap()
    d["b2"] = nc.dram_tensor("b2", [L, D], F32, kind="ExternalInput").ap()
    d["blm"] = nc.dram_tensor("blm", [VL], F32, kind="ExternalInput").ap()
    d["out"] = nc.dram_tensor("logits", [T, VL], BF16, kind="ExternalOutput").ap()

    with tile.TileContext(nc) as tc, ExitStack() as ctx:
        _body(ctx, tc, bias_flags, d)
    nc.compile()
    return nc


def _body(ctx, tc, bf, d):
    nc = tc.nc
    pool = lambda name, bufs, **kw: ctx.enter_context(
        tc.tile_pool(name=name, bufs=bufs, **kw))

    const = pool("const", 1)
    ln_p = pool("ln", 4)
    lnh_p = pool("lnh", 8)
    sq_p = pool("sq", 2)
    x_p = pool("x", 1)
    hT_p = pool("hT", 2)
    lmw_p = pool("lmw", 2)
    lmo_p = pool("lmo", 4)
    dram = pool("dram", 16, space="DRAM")

    q_p = pool("q", 4)
    k_p = pool("k", 6)
    v_p = pool("v", 2)
    es_p = pool("es", 2)
    oT_p = pool("oT", 8)
    rdn_p = pool("rdn", 4)
    y_p = pool("y", 2)
    um_p = pool("um", 3)
    wqkv_p = pool("wqkv", 8)
    wo_p = pool("wo", 2)
    w1_p = pool("w1", 7)
    w2_p = pool("w2", 3)
    bias_p = pool("bias", 2)

    # PSUM: 8 banks total
    sc_p = pool("sc", 2, space="PSUM")       # [128, 2, 512] f32 = 2 banks each
    po_p = pool("po", 2, space="PSUM")       # [128, 512] 1 bank each
    misc_p = pool("misc", 2, space="PSUM")   # [128, 512] 1 bank each

    # constants
    ident = const.tile([P, P], BF16)
    make_identity(nc, ident)
    magic4 = const.tile([P, 4], mybir.dt.int32)
    nc.vector.memset(magic4, 0x5F3759DF)
    # pre-warm the ACT table set (exp/ln/identity/square share one set)
    warm = const.tile([P, 1], F32)
    nc.scalar.activation(warm, magic4[:, 0:1].bitcast(F32), AF.Exp, scale=0.0)
    ones64 = const.tile([P, HD], F32)
    nc.vector.memset(ones64, 1.0)
    epsc = const.tile([P, 1], F32)
    nc.vector.memset(epsc, 1e-5)
    # causal masking for the diagonal 128-chunk, folded into the scores
    # matmul: triA[k, j] = -1e9 where k < j else 0, so triA.T @ ident
    # accumulates -1e9 onto strictly-upper (j > t') score positions and
    # exp underflows them to exact zeros — no separate mask op.
    triA = const.tile([P, P], BF16)
    nc.vector.memset(triA, -1e9)
    nc.gpsimd.affine_select(
        out=triA, in_=triA, compare_op=ALU.is_ge, fill=0.0,
        base=-1, channel_multiplier=-1, pattern=[[1, P]])
    ones_row = None
    if any(bf.values()):
        ones_row = const.tile([1, P], F32)
        nc.vector.memset(ones_row, 1.0)

    x_sb = x_p.tile([P, NT, D], F32)

    # ---------------- LN chains ----------------
    # rstd = exp(-0.5 * ln(var + eps)); var from ACT accumulate passes.

    def chain_act(half):
        """Full-ACT LN chain for `half`: use when the chain's execution
        window has an idle ACT (ffn / lm entry windows)."""
        s4 = ln_p.tile([P, 2, 4], F32, tag="s4")
        negm4 = ln_p.tile([P, 4], F32, tag="negm4")
        for tcl in range(4):
            xc = x_sb[:, half * 4 + tcl, :]
            junk = sq_p.tile([P, D], BF16, tag="junk", name="junk")
            nc.scalar.activation(junk, xc, AF.Identity,
                                 accum_out=s4[:, 0, tcl:tcl + 1])
            nc.scalar.mul(negm4[:, tcl:tcl + 1], s4[:, 0, tcl:tcl + 1],
                          -1.0 / D)
            junk2 = sq_p.tile([P, D], BF16, tag="junk", name="junk2")
            nc.scalar.activation(junk2, xc, AF.Square,
                                 bias=negm4[:, tcl:tcl + 1],
                                 accum_out=s4[:, 1, tcl:tcl + 1])
        l4 = ln_p.tile([P, 4], F32, tag="l4")
        nc.scalar.activation(l4, s4[:, 1, :], AF.Ln, scale=1.0 / D, bias=epsc)
        rstd4 = ln_p.tile([P, 4], F32, tag="rstd4")
        nc.scalar.activation(rstd4, l4, AF.Exp, scale=-0.5)
        nm4 = ln_p.tile([P, 4], F32, tag="nm4")
        for tcl in range(4):
            nc.scalar.mul(nm4[:, tcl:tcl + 1], rstd4[:, tcl:tcl + 1],
                          negm4[:, tcl:tcl + 1])
        hts = []
        for tcl in range(4):
            h = lnh_p.tile([P, D], BF16, tag="h")
            nc.scalar.activation(
                h, x_sb[:, half * 4 + tcl, :], AF.Identity,
                bias=nm4[:, tcl:tcl + 1], scale=rstd4[:, tcl:tcl + 1])
            hts.append(h)
        return hts

    def chain_stats(half):
        """DVE bn_stats part of the hybrid chain (emit where DVE drains
        early relative to the accum feeding it)."""
        mv4 = ln_p.tile([P, 2, 4], F32, tag="mv4")
        for tcl in range(4):
            xc = x_sb[:, half * 4 + tcl, :]
            st = ln_p.tile([P, 2, 6], F32, tag="st")
            for s in range(2):
                nc.vector.bn_stats(st[:, s, :], xc[:, s * 384:(s + 1) * 384])
            nc.vector.bn_aggr(mv4[:, :, tcl], st)
        return mv4

    def chain_tail(mv4, half):
        """ACT tail of the hybrid chain."""
        l4 = ln_p.tile([P, 4], F32, tag="l4")
        nc.scalar.activation(l4, mv4[:, 1, :], AF.Ln, bias=epsc)
        rstd4 = ln_p.tile([P, 4], F32, tag="rstd4")
        nc.scalar.activation(rstd4, l4, AF.Exp, scale=-0.5)
        negm4 = ln_p.tile([P, 4], F32, tag="negm4")
        nc.scalar.mul(negm4, mv4[:, 0, :], -1.0)
        nm4 = ln_p.tile([P, 4], F32, tag="nm4")
        for tcl in range(4):
            nc.scalar.mul(nm4[:, tcl:tcl + 1], rstd4[:, tcl:tcl + 1],
                          negm4[:, tcl:tcl + 1])
        hts = []
        for tcl in range(4):
            h = lnh_p.tile([P, D], BF16, tag="h")
            nc.scalar.activation(
                h, x_sb[:, half * 4 + tcl, :], AF.Identity,
                bias=nm4[:, tcl:tcl + 1], scale=rstd4[:, tcl:tcl + 1])
            hts.append(h)
        return hts

    def ln_transpose(hts):
        hT = hT_p.tile([P, KD, TQH], BF16, tag="hT")
        for kt in range(KD):
            pt = misc_p.tile([P, TQH], F32, tag="misc", name="ptb").bitcast(BF16)
            for tcl in range(4):
                nc.tensor.transpose(
                    pt[:, tcl * P:(tcl + 1) * P],
                    hts[tcl][:, kt * P:(kt + 1) * P], ident)
            nc.vector.tensor_copy(hT[:, kt, :], pt[:, 0:TQH])
        return hT

    def bias_mm(psum_ap, brow_ap):
        # += ones^T @ brow : K=1 matmul accumulating a broadcast row vector
        nc.tensor.matmul(psum_ap, _r(ones_row), _r(brow_ap),
                         start=False, stop=False)

    # ---------------- layer weight loads ----------------
    wqkv_sbs = [None] * L
    wo_sbs = [None] * L
    w1_sbs = [None] * L
    bias_sbs = [None] * L

    def load_wqkv(l):
        ws = []
        for kt in range(KD):
            w = wqkv_p.tile([P, 3 * DL], BF16, tag="wqkv")
            nc.sync.dma_start(w, d["wqkv"][l, kt * P:(kt + 1) * P, :])
            ws.append(w)
        wqkv_sbs[l] = ws
        bqk_sb = brow_v = brow_o = brow_2 = b1_sb = None
        if bf["qk"]:
            bqk_sb = bias_p.tile([P, 6], F32, tag="bqk")
            nc.sync.dma_start(
                bqk_sb,
                d["bqkv"][l, 0:2 * DL].rearrange("(w q p) -> p (w q)", p=P, w=2))
        if bf["v"]:
            brow_v = bias_p.tile([1, DL], F32, tag="bv")
            nc.sync.dma_start(brow_v, d["bqkv"][l, 2 * DL:3 * DL][None, :])
        if bf["o"]:
            brow_o = bias_p.tile([1, D], F32, tag="bo")
            nc.sync.dma_start(brow_o, d["bo"][l][None, :])
        if bf["b1"]:
            b1_sb = bias_p.tile([P, 12], F32, tag="b1")
            nc.sync.dma_start(b1_sb, d["b1"][l].rearrange("(m p) -> p m", p=P))
        if bf["b2"]:
            brow_2 = bias_p.tile([1, D], F32, tag="b2")
            nc.sync.dma_start(brow_2, d["b2"][l][None, :])
        bias_sbs[l] = (bqk_sb, brow_v, brow_o, brow_2, b1_sb)

    def load_wo_w1(l):
        wo_sb = wo_p.tile([P, LH // 2, D], BF16, tag="wo")
        nc.sync.dma_start(wo_sb, d["wo"][l])
        wo_sbs[l] = wo_sb
        ws = []
        for kt in range(KD):
            w = w1_p.tile([P, F1], BF16, tag="w1")
            nc.sync.dma_start(w, d["w1"][l, kt * P:(kt + 1) * P, :])
            ws.append(w)
        w1_sbs[l] = ws

    # ---------------- transformer layers ----------------
    qT, kT, v_sb, oT = {}, {}, {}, {}

    def qkv_section(l, half, hT):
        wqkv_sb = wqkv_sbs[l]
        bqk_sb, brow_v = bias_sbs[l][0], bias_sbs[l][1]
        for pair in range(3):
            for which, store, pp in ((0, qT, q_p), (1, kT, k_p)):
                dst = pp.tile([P, TQH], BF16, tag="qkT")
                ps = misc_p.tile([P, TQH], F32, tag="misc")
                for kt in range(KD):
                    nc.tensor.matmul(
                        ps,
                        wqkv_sb[kt][:, which * DL + pair * P:
                                    which * DL + (pair + 1) * P],
                        hT[:, kt, :],
                        start=(kt == 0), stop=(kt == KD - 1))
                if bf["qk"]:
                    nc.scalar.activation(
                        dst, ps, AF.Identity,
                        bias=bqk_sb[:, which * 3 + pair:which * 3 + pair + 1])
                else:
                    nc.vector.tensor_copy(dst, ps)
                store[(pair, half)] = dst
        # v natural [j, head, d'] + ones column, bf16
        vt = v_p.tile([P, 4, LH, HD + 1], BF16, tag="v")
        nc.vector.memset(vt[:, :, :, HD:HD + 1], 1.0)
        for jcl in range(4):
            ps = misc_p.tile([P, TQH], F32, tag="misc")
            for kt in range(KD):
                nc.tensor.matmul(
                    ps[:, 0:DL], hT[:, kt, jcl * P:(jcl + 1) * P],
                    wqkv_sb[kt][:, 2 * DL:3 * DL],
                    start=(kt == 0), stop=(kt == KD - 1))
            if bf["v"]:
                bias_mm(ps[:, 0:DL], brow_v)
            nc.vector.tensor_copy(
                vt[:, jcl, :, 0:HD],
                ps[:, 0:DL].rearrange("p (h e) -> p h e", h=LH))
        v_sb[half] = vt

    def pairs_section(half):
        # scores -> exp (merged across head pair) -> PV -> normalize
        njt = 4 * (half + 1)
        for pair in range(3):
            es = es_p.tile([P, 8, 2, TQH], BF16, tag="es")
            for jt in range(njt):
                lst = max(0, jt * P - half * TQH)
                doff = jt * P - half * TQH
                diag = doff >= 0
                sctile = sc_p.tile([P, 2, TQH], F32, tag="sc")
                for hh in range(2):
                    nc.tensor.matmul(
                        sctile[:, hh, lst:],
                        kT[(pair, jt // 4)][hh * HD:(hh + 1) * HD,
                                            (jt % 4) * P:(jt % 4 + 1) * P],
                        qT[(pair, half)][hh * HD:(hh + 1) * HD, lst:],
                        start=True, stop=not diag)
                if diag:
                    for hh in range(2):
                        nc.tensor.matmul(
                            sctile[:, hh, doff:doff + P],
                            triA, ident, start=False, stop=True)
                nc.scalar.activation(es[:, jt, :, lst:], sctile[:, :, lst:],
                                     AF.Exp, scale=0.125)
            ot = oT_p.tile([P, TQH], BF16, tag="oT")
            ptb2 = misc_p.tile([P, TQH], F32, tag="misc",
                               name="ptm2").bitcast(BF16)
            for hh in range(2):
                lh = pair * 2 + hh
                po = po_p.tile([P, TQH], F32, tag="po")
                for jt in range(njt):
                    lst = max(0, jt * P - half * TQH)
                    nc.tensor.matmul(
                        po[0:HD + 1, lst:],
                        v_sb[jt // 4][:, jt % 4, lh, :],
                        es[:, jt, hh, lst:],
                        start=(jt == 0), stop=(jt == njt - 1))
                # normalize per token via transpose round-trip (bf16),
                # batched over the 4 token chunks.
                oT65 = rdn_p.tile([HD + 1, TQH], BF16, tag="oT65")
                nc.vector.tensor_copy(oT65, po[0:HD + 1, :])
                ptb = misc_p.tile([P, TQH], F32, tag="misc",
                                  name="ptm").bitcast(BF16)
                # stride 68 keeps each chunk's PSUM offset 4B-aligned
                ptv = ptb[:, 0:4 * 68].rearrange("p (a b) -> p a b", a=4)
                for tcl in range(4):
                    nc.tensor.transpose(
                        ptv[:, tcl, 0:HD + 1],
                        oT65[:, tcl * P:(tcl + 1) * P],
                        ident[0:HD + 1, 0:HD + 1])
                rc4 = ln_p.tile([P, 4], F32, tag="rc4")
                nc.vector.reciprocal(rc4, ptv[:, :, HD])
                on4 = rdn_p.tile([P, 4, HD], BF16, tag="on4")
                nc.vector.tensor_mul(
                    on4, ptv[:, :, 0:HD],
                    rc4[:, :, None].broadcast_to((P, 4, HD)))
                h0 = hh * HD
                for tcl in range(4):
                    nc.tensor.transpose(
                        ptb2[h0:h0 + HD, tcl * P:(tcl + 1) * P],
                        on4[:, tcl, :], ident)
            nc.vector.tensor_copy(ot, ptb2[:, 0:TQH])
            oT[pair] = ot

    AR_DT = BF16 if AR_BF16 else F32

    def do_allreduce(b_in, b_out, half):
        nc.gpsimd.collective_compute(
            "AllReduce", ALU.add,
            replica_groups=[[0, 1], [2, 3], [4, 5], [6, 7]],
            ins=[b_in.opt()], outs=[b_out.opt()])
        nc.gpsimd.dma_start(
            out=x_sb[:, half * 4:half * 4 + 4, :],
            in_=b_out.rearrange("(n p) t -> p n t", p=P),
            accum_op=ALU.add)

    def oproj_ar(l, half):
        # O-projection -> bounce -> AllReduce -> x += result
        wo_sb = wo_sbs[l]
        brow_o = bias_sbs[l][2]
        b_in = dram.tile([TQH, D], AR_DT, tag="bnc", name="b_in")
        b_out = dram.tile([TQH, D], AR_DT, tag="bnc", name="b_out")
        for tcl in range(4):
            py = sc_p.tile([P, 2, TQH], F32, tag="sc")
            pyf = py.rearrange("p a b -> p (a b)")
            for pairi in range(3):
                for n0, nw in ((0, 512), (512, 256)):
                    nc.tensor.matmul(
                        pyf[:, n0:n0 + nw],
                        oT[pairi][:, tcl * P:(tcl + 1) * P],
                        wo_sb[:, pairi, n0:n0 + nw],
                        start=(pairi == 0), stop=(pairi == 2))
            if bf["o"]:
                for n0, nw in ((0, 512), (512, 256)):
                    bias_mm(pyf[:, n0:n0 + nw], brow_o[:, n0:n0 + nw])
            ysb = y_p.tile([P, D], AR_DT, tag="y")
            nc.vector.tensor_copy(ysb, pyf[:, 0:D])
            nc.sync.dma_start(b_in[tcl * P:(tcl + 1) * P, :], ysb)
        do_allreduce(b_in, b_out, half)

    def ffn_half(l, half, hT2):
        w1_sb = w1_sbs[l]
        brow_2, b1_sb = bias_sbs[l][3], bias_sbs[l][4]
        b_in = dram.tile([TQH, D], AR_DT, tag="bnc", name="b_in")
        b_out = dram.tile([TQH, D], AR_DT, tag="bnc", name="b_out")
        for quarter in range(2):
            py0 = sc_p.tile([P, 2, TQH], F32, tag="sc")
            py1 = sc_p.tile([P, 2, TQH], F32, tag="sc")
            pyfs = [py0.rearrange("p a b -> p (a b)"),
                    py1.rearrange("p a b -> p (a b)")]
            for m in range(12):
                pu = po_p.tile([P, TQH], F32, tag="po")
                for kt in range(KD):
                    nc.tensor.matmul(
                        pu[:, 0:256], w1_sb[kt][:, m * P:(m + 1) * P],
                        hT2[:, kt, quarter * 256:(quarter + 1) * 256],
                        start=(kt == 0), stop=(kt == KD - 1))
                um = um_p.tile([P, 256], BF16, tag="uT")
                if bf["b1"]:
                    nc.vector.tensor_scalar(
                        um, pu[:, 0:256], b1_sb[:, m:m + 1], 0.0,
                        op0=ALU.add, op1=ALU.max)
                else:
                    nc.vector.tensor_scalar_max(um, pu[:, 0:256], 0.0)
                w2m = w2_p.tile([P, D], BF16, tag="w2")
                nc.sync.dma_start(w2m, d["w2"][l, m * P:(m + 1) * P, :])
                for t2 in range(2):
                    for n0, nw in ((0, 512), (512, 256)):
                        nc.tensor.matmul(
                            pyfs[t2][:, n0:n0 + nw],
                            um[:, t2 * P:(t2 + 1) * P],
                            w2m[:, n0:n0 + nw],
                            start=(m == 0), stop=(m == 11))
            for t2 in range(2):
                if bf["b2"]:
                    for n0, nw in ((0, 512), (512, 256)):
                        bias_mm(pyfs[t2][:, n0:n0 + nw], brow_2[:, n0:n0 + nw])
                ysb = y_p.tile([P, D], AR_DT, tag="y")
                nc.vector.tensor_copy(ysb, pyfs[t2][:, 0:D])
                tcl = quarter * 2 + t2
                nc.sync.dma_start(b_in[tcl * P:(tcl + 1) * P, :], ysb)
        do_allreduce(b_in, b_out, half)

    # ---------------- lm_head helpers ----------------
    brow_lm = None
    if bf["lm"]:
        brow_lm = lmo_p.tile([1, VL], F32, tag="blm")
        nc.sync.dma_start(brow_lm, d["blm"][None, :])
    nvt = (VL + 511) // 512
    hfT = [None, None]

    def lm_tile(vt, wt, tcgs, dve_only=False):
        v0 = vt * 512
        vw = min(512, VL - v0)
        for tcg in tcgs:
            half, tcl = tcg // 4, tcg % 4
            pl = misc_p.tile([P, 512], F32, tag="misc", name="pl")
            for kt in range(KD):
                nc.tensor.matmul(
                    pl[:, 0:vw],
                    hfT[half][:, kt, tcl * P:(tcl + 1) * P],
                    wt[:, kt, 0:vw],
                    start=(kt == 0), stop=(kt == KD - 1))
            if bf["lm"]:
                bias_mm(pl[:, 0:vw], brow_lm[:, v0:v0 + vw])
            lo = lmo_p.tile([P, 512], BF16, tag="lmo")
            if dve_only or tcg % 2 == 1:
                nc.vector.tensor_copy(lo[:, 0:vw], pl[:, 0:vw])
            else:
                nc.scalar.activation(lo[:, 0:vw], pl[:, 0:vw], AF.Copy)
            nc.sync.dma_start(
                d["out"][tcg * P:(tcg + 1) * P, v0:v0 + vw], lo[:, 0:vw])

    def lm_wt(vt):
        v0 = vt * 512
        vw = min(512, VL - v0)
        wt = lmw_p.tile([P, KD, 512], BF16, tag="lmw")
        nc.sync.dma_start(
            wt[:, :, 0:vw],
            d["wlm"][:, v0:v0 + vw].rearrange("(k p) w -> p k w", p=P))
        return wt

    # ---------------- emission ----------------
    # layer-0 LN(h) comes precomputed+pretransposed from the host: the
    # startup chain+transpose serial latency disappears entirely. Its DMA
    # goes first (with wqkv) so qkv(0) starts as early as possible; the
    # raw residual is only needed by the f0 chain, ~60us in.
    hT0in = [None, None]
    t_ = hT_p.tile([P, KD, TQH], BF16, tag="hT", name="hT0in0")
    nc.sync.dma_start(t_, d["h0T"][:, :, 0:TQH])
    hT0in[0] = t_
    load_wqkv(0)
    t_ = hT_p.tile([P, KD, TQH], BF16, tag="hT", name="hT0in1")
    nc.sync.dma_start(t_, d["h0T"][:, :, TQH:T])
    hT0in[1] = t_
    x0v = d["x0"].rearrange("(n p) t -> p n t", p=P)
    nc.sync.dma_start(x_sb[:, 0:4, :], x0v[:, 0:4, :])
    nc.sync.dma_start(x_sb[:, 4:8, :], x0v[:, 4:8, :])
    load_wo_w1(0)
    hts_a0 = None
    for l in range(L):
        hTa0 = hT0in[0] if l == 0 else ln_transpose(hts_a0)
        qkv_section(l, 0, hTa0)
        if l > 0:
            mv_a1 = chain_stats(1)  # dep: accum(f1^{l-1})
        pairs_section(0)
        if l > 0:
            hts_a1 = chain_tail(mv_a1, 1)
        oproj_ar(l, 0)
        hTa1 = hT0in[1] if l == 0 else ln_transpose(hts_a1)
        qkv_section(l, 1, hTa1)
        if l < L - 1:
            load_wqkv(l + 1)       # prefetch next layer's QKV weights
        else:
            wts01 = [lm_wt(0), lm_wt(1)]
        pairs_section(1)
        oproj_ar(l, 1)
        mv_f0 = chain_stats(0)     # dep: accum(a0^l); after ysb(a1) casts
        hts_f0 = chain_tail(mv_f0, 0)
        hts_f1 = chain_act(1)      # dep: accum(a1^l); runs during ffn(0)
        hTf0 = ln_transpose(hts_f0)
        ffn_half(l, 0, hTf0)
        hts_a0 = chain_act(0)      # dep: accum(f0^l); next layer / lm half0
        if l < L - 1:
            load_wo_w1(l + 1)      # prefetch next layer's wo/w1
        hTf1 = ln_transpose(hts_f1)
        ffn_half(l, 1, hTf1)

    # ---------------- final LN + lm_head ----------------
    hts_lm1 = chain_act(1)         # dep: accum(f1^{L-1}); runs under lm half0
    hfT[0] = ln_transpose(hts_a0)
    for vt in range(LM_FIRST):
        wt = wts01[vt] if vt < 2 else lm_wt(vt)
        lm_tile(vt, wt, range(4), dve_only=True)
    hfT[1] = ln_transpose(hts_lm1)
    for vt in range(LM_FIRST):
        lm_tile(vt, lm_wt(vt), range(4, 8))
    for vt in range(LM_FIRST, nvt):
        lm_tile(vt, lm_wt(vt), range(8))


# ---------------------------------------------------------------------------
# host side
# ---------------------------------------------------------------------------

_CACHE = {}


def _get_program(bias_flags):
    key = tuple(sorted(bias_flags.items()))
    if key not in _CACHE:
        _CACHE[key] = build_program(bias_flags)
    return _CACHE[key]


def _bf16(a):
    return np.ascontiguousarray(a.astype(ml_dtypes.bfloat16))


def make_in_maps(idx, tok_emb, pos_emb, wq, wk, wv, wo, bo,
                 ln1_g, ln1_b, ln2_g, ln2_b, w1, b1, w2, b2,
                 lnf_g, lnf_b, w_lm, b_lm):
    f = lambda a: np.asarray(a, dtype=np.float32)
    idx = np.asarray(idx)
    tok_emb, pos_emb = f(tok_emb), f(pos_emb)
    wq, wk, wv, wo, bo = f(wq), f(wk), f(wv), f(wo), f(bo)
    ln1_g, ln1_b, ln2_g, ln2_b = f(ln1_g), f(ln1_b), f(ln2_g), f(ln2_b)
    w1, b1, w2, b2 = f(w1), f(b1), f(w2), f(b2)
    lnf_g, lnf_b, w_lm, b_lm = f(lnf_g), f(lnf_b), f(w_lm), f(b_lm)

    # fold LN affine into following matmuls
    wq_f = ln1_g[:, :, None] * wq
    wk_f = ln1_g[:, :, None] * wk
    wv_f = ln1_g[:, :, None] * wv
    bq_f = np.einsum("ld,ldo->lo", ln1_b, wq)
    bk_f = np.einsum("ld,ldo->lo", ln1_b, wk)
    bv_f = np.einsum("ld,ldo->lo", ln1_b, wv)
    w1_f = ln2_g[:, :, None] * w1
    b1_f = b1 + np.einsum("ld,ldo->lo", ln2_b, w1)
    wlm_f = lnf_g[:, None] * w_lm
    blm_f = b_lm + lnf_b @ w_lm

    bias_flags = {
        "qk": bool(np.any(bq_f) or np.any(bk_f)),
        "v": bool(np.any(bv_f)),
        "o": bool(np.any(bo)),
        "b1": bool(np.any(b1_f)),
        "b2": bool(np.any(b2)),
        "lm": bool(np.any(blm_f)),
    }

    in_maps = []
    for c in range(NCORES):
        seq, tp = c // 2, c % 2
        sl = slice(tp * DL, (tp + 1) * DL)
        sf = slice(tp * F1, (tp + 1) * F1)
        sv = slice(tp * VL, (tp + 1) * VL)
        x0 = tok_emb[idx[seq]] + pos_emb[:T]
        m0 = x0.mean(-1, keepdims=True)
        v0 = x0.var(-1, keepdims=True)
        h0 = (x0 - m0) / np.sqrt(v0 + 1e-5)
        h0T = np.ascontiguousarray(
            h0.T.reshape(KD, P, T).transpose(1, 0, 2))
        wqkv_c = np.concatenate(
            [wq_f[:, :, sl], wk_f[:, :, sl], wv_f[:, :, sl]], axis=2)
        wo_c = np.ascontiguousarray(
            wo[:, sl, :].reshape(L, 3, 2, HD, D).transpose(0, 2, 3, 1, 4)
            .reshape(L, P, 3, D))
        bqkv_c = np.concatenate([bq_f[:, sl], bk_f[:, sl], bv_f[:, sl]], axis=1)
        in_maps.append({
            "x0": np.ascontiguousarray(x0, dtype=np.float32),
            "h0T": _bf16(h0T),
            "wqkv": _bf16(wqkv_c),
            "wo": _bf16(wo_c),
            "w1": _bf16(w1_f[:, :, sf]),
            "w2": _bf16(w2[:, sf, :]),
            "wlm": _bf16(wlm_f[:, sv]),
            "bqkv": np.ascontiguousarray(bqkv_c),
            "bo": np.ascontiguousarray(bo if tp == 0 else np.zeros_like(bo)),
            "b1": np.ascontiguousarray(b1_f[:, sf]),
            "b2": np.ascontiguousarray(b2 if tp == 0 else np.zeros_like(b2)),
            "blm": np.ascontiguousarray(blm_f[sv]),
        })
    return in_maps, bias_flags


def assemble(outs):
    logits = np.empty((B, T, V), dtype=np.float32)
    for seq in range(B):
        logits[seq, :, :VL] = np.asarray(outs[2 * seq], dtype=np.float32)
        logits[seq, :, VL:] = np.asarray(outs[2 * seq + 1], dtype=np.float32)
    return logits


def kernel(**inputs):
    in_maps, bias_flags = make_in_maps(**inputs)
    nc = _get_program(bias_flags)
    res = bass_utils.run_bass_kernel_spmd(
        nc, in_maps, core_ids=list(range(NCORES)))
    return assemble([res.results[c]["logits"] for c in range(NCORES)])
